# revision 57
# baseline (speedup 1.0000x reference)
"""MoBA (Mixture of Block Attention) Trainium2 Bass kernel.

Problem: B=1, S=2048, D=2048, H=16 heads (d=128), chunk=256, topk=4.
Sharding: 2 heads per core across 8 cores (tensor-parallel on H).
Each core computes q/k/v projections for its 2 heads (fp32r matmuls to
keep the block-gate at ~FP22 precision), RoPE, per-head block gating
(top-4 of 8 chunks), sparse-masked causal attention computed in
transposed score layout (scoresT[key, q]), RMSNorm, and a partial
output projection o_norm @ Wo[:, head_slice].T.  The host sums the 8
partial (bf16) outputs.

Key optimizations over the first working version:
- The block mask is applied multiplicatively during the PV PSUM drain
  (the mask is per-query there, i.e. per-partition, so one fused
  scalar_tensor_tensor per block does it) instead of accumulating a
  one-hot mask matmul into every score tile: the score pass is one
  matmul per tile and the mask transposes disappear.
- Row sums ride along the PV matmul via a ones-column appended to V.
- Phase B runs as two software-pipelined passes (scores/exp/PV, then
  norm/transpose/out-projection) so Exp and Sqrt never alternate on
  the activation engine (act-table reloads cost 1.3us each); a
  data-dependency fence on the Sqrt's scale operand stops the
  scheduler from hoisting it into the exp pass.
- The in-diagonal causal mask is a post-exp 0/1 multiply on the
  otherwise idle GPSIMD engine; per-head drains/copies are spread
  over DVE/ACT/GPSIMD (GPSIMD cannot touch PSUM).
- Input DMAs are split and ordered so the first projection matmul
  starts ~5us in; the output is written bf16 (halves the out DMA).

Self-contained: hardcodes all shapes; builds and caches one Bass/Tile
program, runs it SPMD on cores 0-7 via run_bass_kernel_spmd.
"""

import math
from contextlib import ExitStack

import numpy as np
import ml_dtypes

S = 2048
D = 2048
H = 16
DH = 128          # head dim
CHUNK = 256
NBLK = S // CHUNK  # 8
NCORES = 8
HLOC = H // NCORES  # 2 heads per core
FLOC = HLOC * DH    # 256 features per core
P = 128
NT = S // P         # 16 query chunks of 128
SM_SCALE = DH ** -0.5
NEG = -1.0e30
POS = 1.0e30
NEGBIG = -3.0e30
EPS = 1e-6
THETA = 10000.0
DHP = DH + 1        # v columns + ones column (row sums)

_CACHE = {}


def _exp_pieces(width):
    """Split a score slab into <=1024-wide exp pieces (2 PSUM banks per
    piece -> one activation instruction), themselves split into <=512
    matmuls that keep every fp32r output >=256 wide (1 cycle/row) and
    inside one PSUM bank; only the final 128-wide slab is narrower."""
    pieces = []
    off = 0
    rem = width
    while rem > 0:
        if rem >= 1280 or rem == 1024:
            p = 1024
        elif rem > 1024:
            p = rem - 256          # 768..1023
        elif rem >= 768:
            p = 512
        elif rem > 512:
            p = rem - 256          # 257..511
        else:
            p = rem
        mms = [512, p - 512] if p > 512 else [p]
        pieces.append((off, p, mms))
        off += p
        rem -= p
    return pieces


def _build_program():
    import concourse.bacc as bacc
    import concourse.tile as tile
    from concourse import mybir

    f32 = mybir.dt.float32
    f32r = mybir.dt.float32r
    bf16 = mybir.dt.bfloat16
    X = mybir.AxisListType
    AF = mybir.ActivationFunctionType
    OP = mybir.AluOpType

    nc = bacc.Bacc("TRN2", target_bir_lowering=False, debug=False)

    def din(name, shape, dt):
        return nc.dram_tensor(name, shape, dt, kind="ExternalInput").ap()

    hT_d = din("hT", [D, S], f32r)          # hidden transposed [D, S]
    wq_d = din("wq", [D, FLOC], f32r)       # Wq[hs,:].T
    wk_d = din("wk", [D, FLOC], f32r)
    wv_d = din("wv", [D, FLOC], f32r)
    wo_d = din("wo", [FLOC, D], bf16)      # (Wo[:,hs] * w).T
    cos2_d = din("cos2", [P, S], f32)      # [cosT; cosT]
    sin2_d = din("sin2", [P, S], f32)      # [-sinT; sinT]
    triT_d = din("triT", [P, P], bf16)     # tri01[i,j] = 1 if i<=j else 0
    pswap_d = din("pswap", [P, P], f32r)   # half-swap permutation
    oneh_d = din("oneh", [NBLK, NBLK * P], bf16)  # oneh[k, b*128+m] = (k==b)
    id_d = din("ident", [P, P], bf16)      # identity for PE transpose (bf16)
    mulM_d = din("mulM", [P, P], f32)      # gate mult mask  [*, t*8+n]
    addM_d = din("addM", [P, P], f32)      # gate add mask
    out_d = nc.dram_tensor("out", [S, D], bf16, kind="ExternalOutput").ap()


    with tile.TileContext(nc) as tc:
        with ExitStack() as ctx:
            const_pool = ctx.enter_context(tc.tile_pool(name="consts", bufs=1))
            qkv_pool = ctx.enter_context(tc.tile_pool(name="qkv", bufs=1))
            tk_pool = ctx.enter_context(tc.tile_pool(name="topk", bufs=1))
            tail_pool = ctx.enter_context(tc.tile_pool(name="tail", bufs=1))
            # 8 PSUM banks: 2x[P,1024] (4) + 1x[P,512] (1) + 3x[P,258] (3)
            ps_big = ctx.enter_context(
                tc.tile_pool(name="psbig", bufs=2, space="PSUM"))
            ps_sm = ctx.enter_context(
                tc.tile_pool(name="pssm", bufs=1, space="PSUM"))
            ps_po = ctx.enter_context(
                tc.tile_pool(name="pspo", bufs=3, space="PSUM"))

            # ---- constants ----
            tri01_t = const_pool.tile([P, P], bf16)
            pswap_t = const_pool.tile([P, P], f32r)
            id_t = const_pool.tile([P, P], bf16)
            mulM_t = const_pool.tile([P, P], f32)
            addM_t = const_pool.tile([P, P], f32)
            negbig_t = const_pool.tile([P, 2 * P], f32)
            zero_t = const_pool.tile([P, 1], f32)
            wo_t = const_pool.tile([P, HLOC, D], bf16)
            # small consts + late-needed consts go on the ACT queue so the
            # SP queue stays dedicated to the phase-A streaming order.
            nc.scalar.dma_start(tri01_t[:], triT_d)
            nc.scalar.dma_start(pswap_t[:], pswap_d)
            nc.scalar.dma_start(id_t[:], id_d)
            nc.scalar.dma_start(mulM_t[:], mulM_d)
            nc.scalar.dma_start(addM_t[:], addM_d)
            nc.scalar.dma_start(wo_t[:], wo_d.rearrange("(fc p) j -> p fc j", p=P))
            nc.vector.memset(negbig_t[:], NEGBIG)
            nc.vector.memset(zero_t[:], 0.0)

            # ---- persistent activations ----
            qT = qkv_pool.tile([P, HLOC, S], f32r)    # roped q, [d, head, s]
            kT = qkv_pool.tile([P, HLOC, S], f32r)
            v_sb = qkv_pool.tile([P, NT, HLOC, DHP], bf16)
            km_t = qkv_pool.tile([P, HLOC, NBLK], f32r)
            gate_sb = qkv_pool.tile([P, HLOC, NT * NBLK], f32)
            M_sb = qkv_pool.tile([P, HLOC, NT * NBLK], f32)
            wv_t = qkv_pool.tile([P, 16, FLOC], f32r)
            fence_t = qkv_pool.tile([P, 1], f32)   # =1/DH, written late in 2a
            nc.vector.memset(v_sb[:, :, :, DH:DHP], 1.0)   # ones col: row sums
            # pin the Exp act-table before any ACT copies run so the only
            # in-flight table switch is Exp->Sqrt at the output-pass start
            dummy_t = const_pool.tile([P, 1], f32)
            nc.scalar.activation(dummy_t[:], zero_t[:], AF.Exp)

            hT_r = hT_d.rearrange("(dc p) s -> p dc s", p=P)

            # ============ phase A: projections + rope (per 256-token tile) ===
            with ExitStack() as actx:
                w_pool = actx.enter_context(tc.tile_pool(name="weights", bufs=1))
                hid_pool = actx.enter_context(tc.tile_pool(name="hid", bufs=3))
                rsc_pool = actx.enter_context(tc.tile_pool(name="ropes", bufs=2))

                wq_t = w_pool.tile([P, 16, FLOC], f32r)
                wk_t = w_pool.tile([P, 16, FLOC], f32r)
                cos2_t = w_pool.tile([P, S], f32)
                sin2_t = w_pool.tile([P, S], f32)
                wq_r = wq_d.rearrange("(dc p) f -> p dc f", p=P)
                wk_r = wk_d.rearrange("(dc p) f -> p dc f", p=P)
                wv_r = wv_d.rearrange("(dc p) f -> p dc f", p=P)

                ht_tiles = [None] * 8
                ht_tiles[0] = hid_pool.tile([P, 16, 256], f32r, tag="hid",
                                            name="ht0")

                # SP-queue issue order: first-needed first, finely split so
                # the first q chain starts ~4us in; cos/sin slivers ride
                # between hidden chunks just ahead of their rope use.
                for i in range(4):
                    nc.sync.dma_start(wq_t[:, 4 * i:4 * i + 4, :],
                                      wq_r[:, 4 * i:4 * i + 4, :])
                    nc.sync.dma_start(ht_tiles[0][:, 4 * i:4 * i + 4, :],
                                      hT_r[:, 4 * i:4 * i + 4, 0:256])
                nc.sync.dma_start(wk_t[:, 0:8, :], wk_r[:, 0:8, :])
                nc.sync.dma_start(wk_t[:, 8:16, :], wk_r[:, 8:16, :])

                def dma_cs(st):
                    sl = slice(st * 256, (st + 1) * 256)
                    nc.sync.dma_start(cos2_t[:, sl], cos2_d[:, sl])
                    nc.sync.dma_start(sin2_t[:, sl], sin2_d[:, sl])

                def dma_ht(st):
                    if st >= 6:
                        t_ = tail_pool.tile([P, 16, 256], f32r,
                                            name=f"httail{st}")
                    else:
                        t_ = hid_pool.tile([P, 16, 256], f32r, tag="hid",
                                           name=f"ht{st}")
                    ht_tiles[st] = t_
                    sl = slice(st * 256, (st + 1) * 256)
                    nc.sync.dma_start(t_[:, 0:8, :], hT_r[:, 0:8, sl])
                    nc.sync.dma_start(t_[:, 8:16, :], hT_r[:, 8:16, sl])

                dma_cs(0)
                dma_ht(1)
                nc.sync.dma_start(wv_t[:, 0:8, :], wv_r[:, 0:8, :])
                nc.sync.dma_start(wv_t[:, 8:16, :], wv_r[:, 8:16, :])
                dma_cs(1)
                for st in range(2, 8):
                    dma_ht(st)
                    dma_cs(st)

                def emit_v(st, scs=(0, 1)):
                    for sc in scs:
                        pv = ps_po.tile([P, 2 * DHP], f32, tag="po", name="pv")
                        for dc in range(16):
                            nc.tensor.matmul(
                                pv[:, 0:FLOC],
                                lhsT=ht_tiles[st][:, dc, sc * P:(sc + 1) * P],
                                rhs=wv_t[:, dc, :],
                                start=(dc == 0),
                                stop=(dc == 15),
                            )
                        # [tok, (h d)] -> v_sb[tok, chunk, h, d]
                        nc.vector.tensor_copy(
                            v_sb[:, st * 2 + sc, :, 0:DH],
                            pv[:, 0:FLOC].rearrange("p (h d) -> p h d", d=DH))

                def emit_rope(st):
                    # q/k <- q*cos + swap(q)*sin2 where sin2 = [-sin; sin];
                    # the half-swap crosses partitions, so it runs as a PE
                    # permutation matmul (DVE cross-partition reads are
                    # rejected by the backend verifier)
                    sl = slice(st * 256, (st + 1) * 256)
                    for dst in (qT, kT):
                        for hh in range(HLOC):
                            psw = ps_po.tile([P, 2 * DHP], f32, tag="po",
                                             name="psw")
                            nc.tensor.matmul(
                                psw[:, 0:256],
                                lhsT=pswap_t[:],
                                rhs=dst[:, hh, sl],
                                start=True, stop=True,
                            )
                            rs = rsc_pool.tile([P, 256], f32, tag="rope")
                            nc.vector.tensor_mul(rs[:], psw[:, 0:256],
                                                 sin2_t[:, sl])
                            nc.vector.tensor_mul(dst[:, hh, sl], dst[:, hh, sl],
                                                 cos2_t[:, sl])
                            nc.vector.tensor_add(dst[:, hh, sl], dst[:, hh, sl],
                                                 rs[:])
                    # incremental block sums (ranking-equivalent to means)
                    with nc.allow_low_precision(reason="km written as fp32r"):
                        for hh in range(HLOC):
                            nc.vector.reduce_sum(km_t[:, hh, st:st + 1],
                                                 kT[:, hh, sl], axis=X.X)

                for st in range(8):
                    sl = slice(st * 256, (st + 1) * 256)
                    ht = ht_tiles[st]
                    # v first: frees the oldest hid-ring slot early so the
                    # next hidden-chunk DMA can start a full iteration ahead
                    if 2 <= st <= 6:
                        emit_v(st - 2)
                    for wt, dst in ((wq_t, qT), (wk_t, kT)):
                        for fc in range(HLOC):
                            pq = ps_po.tile([P, 2 * DHP], f32, tag="po")
                            for dc in range(16):
                                nc.tensor.matmul(
                                    pq[:, 0:256],
                                    lhsT=wt[:, dc, fc * P:(fc + 1) * P],
                                    rhs=ht[:, dc, :],
                                    start=(dc == 0),
                                    stop=(dc == 15),
                                )
                            if fc == 0:
                                nc.vector.tensor_copy(dst[:, fc, sl],
                                                      pq[:, 0:256])
                            else:
                                nc.scalar.copy(dst[:, fc, sl], pq[:, 0:256])
                    if st >= 1:
                        emit_rope(st - 1)
                emit_rope(7)
                emit_v(5)   # PE work that hides rope(7)/km on the DVE

                # gate + top-k are emitted inside pass 2a (after slab 0) so
                # the PE never waits on the DVE here; v(6)/v(7) fill the
                # exp-heavy early iterations of pass 2a.
                G = HLOC * NT  # 32 groups of 8 blocks
                gw_t = tk_pool.tile([P, G * NBLK], f32)
                lt_t = tk_pool.tile([P, G * NBLK], mybir.dt.int32)
                m_t = tk_pool.tile([P, G], f32)

                def emit_gate():
                    for hh in range(HLOC):
                        pg = ps_po.tile([P, 2 * DHP], f32, tag="po", name="pg")
                        for t in range(NT):
                            nc.tensor.matmul(
                                pg[:, t * NBLK:(t + 1) * NBLK],
                                lhsT=qT[:, hh, t * P:(t + 1) * P],
                                rhs=km_t[:, hh, :],
                                start=True, stop=True,
                            )
                        nc.vector.tensor_mul(gate_sb[:, hh, :], pg[:, 0:P],
                                             mulM_t[:])
                        nc.vector.tensor_add(gate_sb[:, hh, :],
                                             gate_sb[:, hh, :], addM_t[:])

                def emit_topk():
                    gate_f = gate_sb[:].rearrange("p h g -> p (h g)")
                    gw_v = gw_t[:].rearrange("p (g n) -> p g n", n=NBLK)
                    lt_v = lt_t[:].rearrange("p (g n) -> p g n", n=NBLK)
                    nc.vector.tensor_copy(gw_t[:], gate_f)
                    for _ in range(3):
                        nc.vector.reduce_max(m_t[:], gw_v, axis=X.X)
                        mb = m_t[:].rearrange(
                            "p (g o) -> p g o", o=1).to_broadcast((P, G, NBLK))
                        nc.vector.tensor_tensor(lt_v, gw_v, mb, op=OP.is_ge)
                        nc.vector.copy_predicated(gw_t[:], lt_t[:], negbig_t[:])
                    nc.vector.reduce_max(m_t[:], gw_v, axis=X.X)
                    nc.vector.tensor_scalar_max(m_t[:], m_t[:], -1.0e29)
                    mb = m_t[:].rearrange("p (g o) -> p g o", o=1).to_broadcast(
                        (P, G, NBLK))
                    gate_v = gate_sb[:].rearrange("p h (t n) -> p (h t) n",
                                                  n=NBLK)
                    M_v = M_sb[:].rearrange("p h (t n) -> p (h t) n", n=NBLK)
                    nc.vector.tensor_tensor(M_v, gate_v, mb, op=OP.is_ge)

            # ============ phase B: score+PV pass, then norm+output pass =====
            with ExitStack() as bctx:
                att_pool = bctx.enter_context(tc.tile_pool(name="att", bufs=8))
                orow_pool = bctx.enter_context(
                    tc.tile_pool(name="orow", bufs=2))
                pb_pool = bctx.enter_context(tc.tile_pool(name="probs", bufs=1))

                pb_tiles = [[pb_pool.tile([P, S - P * _c], bf16,
                                          name=f"pb{_h}_{_c}",
                                          tag=f"pb{_h}_{_c}")
                             for _c in range(NT)]
                            for _h in range(HLOC)]
                O_sb = pb_pool.tile([P, NT, HLOC, DHP], f32)  # PV out + R
                onT_tiles = {}
                onp_tiles = {}
                sm_toggle = [True]

                QMASK = 16 * P   # queries >= this get the additive mask

                def emit_madd():
                    # additive mask (0 / -1e30) for t>=12, transposed to
                    # [block, q] so a one-hot matmul can add it to scoresT
                    nc.vector.tensor_scalar(
                        Madd_sb[:], M_sb[:, :, 12 * NBLK:],
                        1.0, POS, op0=OP.subtract, op1=OP.mult)
                    for hh in range(HLOC):
                        for tq in range(4):
                            pmt = ps_po.tile([P, 2 * DHP], bf16, tag="po",
                                             name="pmt")
                            nc.tensor.transpose(
                                pmt[:NBLK, 0:P],
                                Madd_sb[:, hh, tq * NBLK:(tq + 1) * NBLK],
                                id_t[:])
                            nc.vector.tensor_copy(
                                MTf_sb[:, hh, tq * P:(tq + 1) * P],
                                pmt[:NBLK, 0:P])

                def emit_scores(c, part="all"):
                    q0 = P * c
                    wl = max(0, min(QMASK - q0, S - q0))
                    for hh in range(HLOC):
                        pbp = pb_tiles[hh][c]
                        pieces = []
                        if part in ("all", "left"):
                            pieces += [(off, p, mms, False)
                                       for off, p, mms in _exp_pieces(wl)]
                        if part in ("all", "right") and S - q0 > wl:
                            pieces += [(wl + off, p, mms, True)
                                       for off, p, mms in
                                       _exp_pieces(S - q0 - wl)]
                        for off, p, mms, masked in pieces:
                            if p > 512:
                                psc = ps_big.tile([P, 1024], f32, tag="big")
                            elif sm_toggle[0]:
                                sm_toggle[0] = False
                                psc = ps_sm.tile([P, 512], f32, tag="sm")
                            else:
                                sm_toggle[0] = True
                                psc = ps_big.tile([P, 1024], f32, tag="big")
                            moff = 0
                            for w in mms:
                                nc.tensor.matmul(
                                    psc[:, moff:moff + w],
                                    lhsT=kT[:, hh, c * P:(c + 1) * P],
                                    rhs=qT[:, hh, q0 + off + moff:
                                           q0 + off + moff + w],
                                    start=True, stop=not masked,
                                )
                                if masked:
                                    m0 = q0 + off + moff - QMASK
                                    nc.tensor.matmul(
                                        psc[:, moff:moff + w],
                                        lhsT=oneh_t[:, c // 2, :],
                                        rhs=MTf_sb[:, hh, m0:m0 + w],
                                        start=False, stop=True,
                                    )
                                moff += w
                            nc.scalar.activation(
                                pbp[:, off:off + p], psc[:, :p], AF.Exp,
                                bias=zero_t[:], scale=SM_SCALE)
                        if part != "right":
                            # causal mask inside the diagonal chunk: multiply
                            # by the 0/1 upper triangle post-exp (idle Pool
                            # engine) instead of a -inf add on the DVE
                            nc.gpsimd.tensor_tensor(
                                pbp[:, 0:P], pbp[:, 0:P], tri01_t[:],
                                op=OP.mult)

                def emit_pv(t):
                    # one PSUM bank tile per causal block, both heads side
                    # by side; blocks for t>=12 were masked additively in
                    # the score pass, so only t=8..11 needs masked drains.
                    if t <= 7 or t >= 16:
                        # all causal blocks selected: one PSUM chain per head
                        po = ps_po.tile([P, 2 * DHP], f32, tag="po",
                                        name="po")
                        for hh in range(HLOC):
                            sl = slice(hh * DHP, hh * DHP + DHP)
                            for c2 in range(t + 1):
                                nc.tensor.matmul(
                                    po[:, sl],
                                    lhsT=pb_tiles[hh][c2][:, P * (t - c2):
                                                          P * (t - c2) + P],
                                    rhs=v_sb[:, c2, hh, :],
                                    start=(c2 == 0), stop=(c2 == t),
                                )
                        nc.vector.tensor_copy(O_sb[:, t, 0, :], po[:, 0:DHP])
                        nc.vector.tensor_copy(O_sb[:, t, 1, :],
                                              po[:, DHP:2 * DHP])
                    else:
                        nb = t // 2 + 1
                        for b in range(nb):
                            po = ps_po.tile([P, 2 * DHP], f32, tag="po",
                                            name="po")
                            c2s = [c2 for c2 in (2 * b, 2 * b + 1) if c2 <= t]
                            for hh in range(HLOC):
                                sl = slice(hh * DHP, hh * DHP + DHP)
                                for i, c2 in enumerate(c2s):
                                    nc.tensor.matmul(
                                        po[:, sl],
                                        lhsT=pb_tiles[hh][c2][:, P * (t - c2):
                                                              P * (t - c2) + P],
                                        rhs=v_sb[:, c2, hh, :],
                                        start=(i == 0),
                                        stop=(i == len(c2s) - 1),
                                    )
                            for hh in range(HLOC):
                                eng = nc.vector
                                O = O_sb[:, t, hh, :]
                                sl = slice(hh * DHP, hh * DHP + DHP)
                                m_ap = M_sb[:, hh, t * NBLK + b:
                                            t * NBLK + b + 1]
                                if b == 0:
                                    eng.tensor_scalar_mul(O, po[:, sl], m_ap)
                                else:
                                    eng.scalar_tensor_tensor(
                                        O, po[:, sl], m_ap, O,
                                        op0=OP.mult, op1=OP.add)

                def emit_norm(t):
                    # o_norm = o * rsqrt(mean(o^2) + eps*R^2) (R = row sum);
                    # division by R and the *o_norm_w fold live in wo.
                    for hh in range(HLOC):
                        O = O_sb[:, t, hh, :]
                        sq = att_pool.tile([P, DH], f32, tag="sq", name="sq")
                        ssa = att_pool.tile([P, 1], f32, tag="ssa", name="ssa")
                        ssb = att_pool.tile([P, 1], f32, tag="ssb", name="ssb")
                        sS = att_pool.tile([P, 1], f32, tag="sS", name="sS")
                        onp = att_pool.tile([P, DH], bf16, tag="onp",
                                            name="onp")
                        nc.vector.scalar_tensor_tensor(
                            sq[:], O[:, 0:DH], 1.0, O[:, 0:DH],
                            op0=OP.mult, op1=OP.mult, accum_out=ssa[:])
                        nc.vector.tensor_scalar(
                            ssb[:], O[:, DH:DHP], O[:, DH:DHP], EPS,
                            op0=OP.mult, op1=OP.mult)
                        # scale carries a dependency on the tail of pass 2a
                        # so the scheduler cannot hoist the Sqrt (and its
                        # act-table switch) into the exp pass; Square/Copy
                        # live in every act-func set and may float freely.
                        nc.scalar.activation(sS[:], ssa[:], AF.Sqrt,
                                             bias=ssb[:], scale=fence_t[:])
                        nc.vector.reciprocal(sS[:], sS[:])
                        nc.gpsimd.tensor_scalar_mul(onp[:], O[:, 0:DH], sS[:])
                        onp_tiles[(t, hh)] = onp

                def emit_mm_head(t):
                    # transposes + onT copies run one iteration ahead of the
                    # output projection so the copies never sit behind the
                    # orow drains in the DVE/ACT queues
                    for hh in range(HLOC):
                        ptr = ps_po.tile([P, 2 * DHP], bf16, tag="po",
                                         name="ptr")
                        nc.tensor.transpose(ptr[:, 0:P],
                                            onp_tiles[(t, hh)][:], id_t[:])
                        onT = att_pool.tile([P, P], bf16, tag="onT",
                                            name="onT")
                        if hh == 0:
                            nc.vector.tensor_copy(onT[:], ptr[:, 0:P])
                        else:
                            nc.scalar.copy(onT[:], ptr[:, 0:P])
                        onT_tiles[(t, hh)] = onT

                def emit_mm(t):
                    orow = orow_pool.tile([P, D], bf16, tag="orow",
                                          name="orow")
                    for half in range(2):
                        pso = ps_big.tile([P, 1024], f32, tag="big",
                                          name="pso")
                        for sub in range(2):
                            nt = half * 2 + sub
                            for hh in range(HLOC):
                                nc.tensor.matmul(
                                    pso[:, sub * 512:(sub + 1) * 512],
                                    lhsT=onT_tiles[(t, hh)][:],
                                    rhs=wo_t[:, hh, nt * 512:(nt + 1) * 512],
                                    start=(hh == 0), stop=(hh == HLOC - 1),
                                )
                        osl = slice(half * 1024, (half + 1) * 1024)
                        if half == 0:
                            nc.vector.tensor_copy(orow[:, osl], pso[:])
                        else:
                            nc.scalar.copy(orow[:, osl], pso[:])
                        nc.sync.dma_start(
                            out_d[t * P:(t + 1) * P, osl], orow[:, osl])

                # ---- pass 2a: scores -> exp -> masked PV, per key chunk;
                # gate/top-k and the deferred v(6)/v(7) chains ride in the
                # exp-heavy early iterations to keep the PE busy.
                for c in range(NT):
                    emit_scores(c, part="left" if c == 0 else "all")
                    if c == 0:
                        emit_gate()
                    if c == 1:
                        emit_scores(0, part="right")
                    if c == NT - 1:
                        # fence: holds 1/DH, data-dependent on t=14 drains so
                        # the Exp->Sqrt table switch fires late in pass 2a,
                        # overlapping the PV(15) matmuls
                        nc.vector.tensor_scalar(
                            fence_t[:], O_sb[:, NT - 2, 1, DH:DHP],
                            0.0, 1.0 / DH, op0=OP.mult, op1=OP.add)
                    emit_pv(c)
                    if c == 0:
                        emit_topk()
                        emit_v(6, scs=(0,))
                    elif c == 1:
                        emit_v(6, scs=(1,))
                    elif c == 2:
                        emit_v(7, scs=(0,))
                    elif c == 3:
                        emit_v(7, scs=(1,))

                # ---- pass 2b: RMSNorm + transpose + output projection ----
                # norm chains lead the matmul pass by two tiles, and the
                # transpose/onT stage leads the projection by one, so the PE
                # never waits on the normalize/copy chains
                emit_norm(0)
                emit_norm(1)
                for t in range(NT):
                    emit_mm_head(t)
                    emit_mm(t)
                    if t + 2 < NT:
                        emit_norm(t + 2)

    nc.compile()
    return nc


def _host_inputs(hidden, Wq, Wk, Wv, Wo, o_norm_w):
    """Build the per-core input maps (host-side sharding + prep)."""
    def fp22_round(x):
        """Round fp32 mantissa to 13 bits (FP22, round-half-to-even) so the
        fp32r TensorEngine path sees exactly these values."""
        u = np.ascontiguousarray(x, dtype=np.float32).view(np.uint32)
        lsb = (u >> np.uint32(10)) & np.uint32(1)
        r = (u + np.uint32(0x1FF) + lsb) & np.uint32(0xFFFFFC00)
        return r.view(np.float32)

    h = np.ascontiguousarray(np.asarray(hidden, dtype=np.float32).reshape(S, D))
    Wq = fp22_round(np.asarray(Wq, dtype=np.float32))
    Wk = fp22_round(np.asarray(Wk, dtype=np.float32))
    Wv = fp22_round(np.asarray(Wv, dtype=np.float32))
    Wo = np.asarray(Wo, dtype=np.float32)
    w = np.asarray(o_norm_w, dtype=np.float32)

    hT = fp22_round(np.ascontiguousarray(h.T))

    pos = np.arange(S, dtype=np.float64)
    inv = 1.0 / (THETA ** (np.arange(0, DH, 2, dtype=np.float64) / DH))
    fr = pos[:, None] * inv[None, :]                # [S, 64]
    cosT = np.cos(fr).T.astype(np.float32)          # [64, S]
    sinT = np.sin(fr).T.astype(np.float32)
    cos2 = np.ascontiguousarray(np.concatenate([cosT, cosT], axis=0))
    sin2 = np.ascontiguousarray(np.concatenate([-sinT, sinT], axis=0))

    # tri01[i, j] = 1 if i <= j else 0   (valid iff query >= key)
    triT = np.where(np.arange(P)[:, None] <= np.arange(P)[None, :],
                    1.0, 0.0).astype(ml_dtypes.bfloat16)
    ident = np.eye(P, dtype=np.float32).astype(ml_dtypes.bfloat16)
    pswap = np.zeros((P, P), dtype=np.float32)
    pswap[(np.arange(P) + 64) % P, np.arange(P)] = 1.0
    oneh = np.zeros((NBLK, NBLK, P), dtype=np.float32)
    for b_ in range(NBLK):
        oneh[b_, b_, :] = 1.0
    oneh = oneh.reshape(NBLK, NBLK * P).astype(ml_dtypes.bfloat16)

    mulM = np.ones((P, P), dtype=np.float32)
    addM = np.zeros((P, P), dtype=np.float32)
    for t in range(NT):
        bq = t // 2
        for n in range(NBLK):
            col = t * NBLK + n
            if n == bq:
                mulM[:, col] = 0.0
                addM[:, col] = POS
            elif n > bq:
                addM[:, col] = NEG

    wtile = np.concatenate([w, w])                  # [256]
    in_maps = []
    for c in range(NCORES):
        hs = slice(FLOC * c, FLOC * (c + 1))
        wq_c = np.ascontiguousarray(Wq[hs, :].T)    # [D, 256]
        wk_c = np.ascontiguousarray(Wk[hs, :].T)
        wv_c = np.ascontiguousarray(Wv[hs, :].T)
        wo_c = np.ascontiguousarray((Wo[:, hs] * wtile[None, :]).T).astype(
            ml_dtypes.bfloat16)                     # [256, D]
        in_maps.append({
            "hT": hT, "wq": wq_c, "wk": wk_c, "wv": wv_c, "wo": wo_c,
            "cos2": cos2, "sin2": sin2, "triT": triT, "ident": ident,
            "pswap": pswap, "oneh": oneh, "mulM": mulM, "addM": addM,
        })
    return in_maps


def get_program():
    if "nc" not in _CACHE:
        _CACHE["nc"] = _build_program()
    return _CACHE["nc"]


def run(inputs, trace=False):
    """Returns (output [1,S,D] float32, BassKernelResults)."""
    from concourse import bass_utils

    in_maps = _host_inputs(
        inputs["hidden_states"], inputs["Wq"], inputs["Wk"],
        inputs["Wv"], inputs["Wo"], inputs["o_norm_w"])
    nc = get_program()
    res = bass_utils.run_bass_kernel_spmd(
        nc, in_maps, core_ids=list(range(NCORES)), trace=trace)
    acc = np.zeros((S, D), dtype=np.float32)
    for r in res.results:
        acc += np.asarray(r["out"], dtype=np.float32)
    return acc.reshape(1, S, D), res


def kernel(**inputs):
    out, _ = run(inputs, trace=False)
    return out
